# revision 12
# baseline (speedup 1.0000x reference)
"""Trainium2 Bass kernel for nn_Dense_BinaryLayer (binary-weight dense layer).

out = x @ Wb + b, where Wb = binarize(W) in {-1, +1}.

Strategy: data-parallel over the 8 NeuronCores — each core handles 2048 rows
of x and the full (replicated) W and b; no collectives.  Host-side prep is
pure data movement: each core's x slice is permuted into per-row-tile
k-major blocks ([it, p, kt, r] layout, so every DMA is one contiguous
256 KiB DRAM read with 2 KiB per-partition segments), and both x and W are
passed as the high 16 bits of each f32 (byte-slice view = bf16 truncation,
no arithmetic; rel err ~3e-3 vs the 2e-2 gate, verified).  fp8 DoubleRow
was tried and measured: per-instruction time on HW equals bf16's, so the
exact 2-pass fp8 split is a wash — single-pass bf16 is the PE floor
(~57.5us busy).

Schedule (the PE stream executes in order, round-robin over 4 PSUM tiles,
so early tiles must land in consumption order):
  - W streams as 8 contiguous per-k-tile chunks spread over BOTH HWDGE
    rings so every wb k-tile is binarized by ~12us; DVE binarizes each
    chunk as it lands (bf16 2x DVE rate).
  - x row-tiles 0/1 go first on the SP HWDGE ring; the rest stream via
    SWDGE (x14/15 on the Act ring to balance), each with a dedicated SBUF
    buffer (no recycling stalls).
  - bf16 matmuls (free dim 512) accumulate in PSUM over the 8 k-tiles;
    DVE adds the broadcast bias on eviction; per-row-tile stores rotate
    across the three DMA rings.
"""
import sys

sys.path.insert(0, "/opt/trn_rl_repo")

import numpy as np

N_TOTAL = 16384
D_IN = 1024
D_OUT = 1024
N_CORES = 8
ROWS = N_TOTAL // N_CORES      # 2048 rows per core
P = 128
K_TILES = D_IN // P            # 8
I_TILES = ROWS // P            # 16
BIN_THRESH = 2.0 ** -24

_cached = {}


def _build():
    import concourse.tile as tile
    from concourse import bacc, mybir

    f32 = mybir.dt.float32
    bf16 = mybir.dt.bfloat16
    TS = mybir.AluOpType

    nc = bacc.Bacc()
    xt_d = nc.declare_dram_parameter(
        "xT", [I_TILES * P, K_TILES * P], bf16, isOutput=False)
    w_d = nc.declare_dram_parameter("W", [D_IN, D_OUT], bf16, isOutput=False)
    b_d = nc.declare_dram_parameter("b", [D_OUT], f32, isOutput=False)
    o_d = nc.declare_dram_parameter("out", [ROWS, D_OUT], f32, isOutput=True)

    with tile.TileContext(nc) as tc:
        with (
            tc.tile_pool(name="const", bufs=1) as const,
            tc.tile_pool(name="wpool", bufs=1) as wpool,
            tc.tile_pool(name="xts", bufs=I_TILES) as xts,
            tc.tile_pool(name="outp", bufs=8) as outp,
            tc.tile_pool(name="pso", bufs=4, space="PSUM") as pso,
        ):
            xt_ap = xt_d[:].rearrange("(it p) (kt r) -> it p kt r", p=P, kt=K_TILES)
            w_ap = w_d[:].rearrange("(kt p) j -> p kt j", p=P)

            # PE warmup: dummy matmuls on a memset tile, issued before any
            # DMA dependency, so the Tensor engine p-state is fully ramped
            # (and its pipeline warm) when the first real matmul is ready
            wsrc = const.tile([P, P], bf16, tag="wsrc")
            nc.gpsimd.memset(wsrc[:], 0.0)
            dummy = pso.tile([P, D_OUT], f32, tag="pso", name="dummy")
            for _ in range(24):
                nc.tensor.matmul(dummy[:, 0:P], wsrc[:], wsrc[:],
                                 start=True, stop=True)

            w_raw = wpool.tile([P, K_TILES, D_OUT], bf16, tag="wraw")
            x_tiles = []
            for it in range(I_TILES):
                x_tiles.append(
                    xts.tile([P, K_TILES, P], bf16, tag="x", name=f"xt_{it}"))

            # earliest-needed data first on each ring (sync starts ~2us
            # before the Act ring, so W0 goes there), W spread over both
            # HWDGE rings so all 8 chunks land by ~12us
            nc.sync.dma_start(w_raw[:, 0, :], w_ap[:, 0, :])
            nc.scalar.dma_start(w_raw[:, 1, :], w_ap[:, 1, :])
            nc.sync.dma_start(x_tiles[0][:], xt_ap[0])
            nc.scalar.dma_start(w_raw[:, 3, :], w_ap[:, 3, :])
            nc.sync.dma_start(w_raw[:, 2, :], w_ap[:, 2, :])
            nc.scalar.dma_start(w_raw[:, 5, :], w_ap[:, 5, :])
            nc.sync.dma_start(x_tiles[1][:], xt_ap[1])
            nc.scalar.dma_start(w_raw[:, 7, :], w_ap[:, 7, :])
            nc.sync.dma_start(w_raw[:, 4, :], w_ap[:, 4, :])
            nc.sync.dma_start(w_raw[:, 6, :], w_ap[:, 6, :])

            # bias broadcast to all partitions
            bb = const.tile([P, D_OUT], f32, tag="bb")
            nc.sync.dma_start(bb[:], b_d[:].unsqueeze(0).partition_broadcast(P))

            # bulk x via SWDGE (in consumption order); last two on Act ring
            for it in range(2, I_TILES - 2):
                nc.gpsimd.dma_start(x_tiles[it][:], xt_ap[it])
            nc.scalar.dma_start(x_tiles[I_TILES - 2][:], xt_ap[I_TILES - 2])
            nc.scalar.dma_start(x_tiles[I_TILES - 1][:], xt_ap[I_TILES - 1])

            # binarize on DVE per k-tile (bf16 => 2x DVE rate):
            # m = (W > c) in {0,1}, then Wb = 2m-1 in {+-1}
            wb = wpool.tile([P, K_TILES, D_OUT], bf16, tag="wb")
            wm = wpool.tile([P, D_OUT], bf16, tag="wm")
            for kt in range(K_TILES):
                nc.vector.tensor_scalar(
                    wm[:], w_raw[:, kt, :], BIN_THRESH, None, TS.is_gt,
                )
                nc.vector.tensor_scalar(
                    wb[:, kt, :], wm[:], 2.0, 1.0, TS.mult, TS.subtract,
                )

            for it in range(I_TILES):
                src = x_tiles[it]
                ps_o = pso.tile([P, D_OUT], f32, tag="pso", name=f"pso_{it}")
                for kt in range(K_TILES):
                    first = kt == 0
                    last = kt == K_TILES - 1
                    nc.tensor.matmul(
                        ps_o[:, 0:512],
                        src[:, kt, :],
                        wb[:, kt, 0:512],
                        start=first, stop=last,
                    )
                    nc.tensor.matmul(
                        ps_o[:, 512:1024],
                        src[:, kt, :],
                        wb[:, kt, 512:1024],
                        start=first, stop=last,
                    )
                # evict + store in halves so the store of half 0 overlaps
                # the eviction of half 1 (shortens the kernel tail)
                out_sb = outp.tile([P, D_OUT], f32, tag="out", name=f"out_{it}")
                for h in range(2):
                    cols = slice(h * 512, (h + 1) * 512)
                    nc.vector.tensor_tensor(
                        out=out_sb[:, cols], in0=ps_o[:, cols],
                        in1=bb[:, cols], op=TS.add,
                    )
                    ring = (nc.sync, nc.scalar, nc.gpsimd)[(2 * it + h) % 3]
                    ring.dma_start(o_d[it * P:(it + 1) * P, cols],
                                   out_sb[:, cols])

    nc.compile()
    nc.finalize()
    return nc


def _hi16(a):
    """bf16 truncation of a C-contiguous f32 array as a byte-slice view."""
    import ml_dtypes

    u = a.view(np.uint16).reshape(*a.shape, 2)[..., 1]
    return np.ascontiguousarray(u).view(ml_dtypes.bfloat16)


def make_in_maps(x, W, b):
    x = np.ascontiguousarray(np.asarray(x, dtype=np.float32))
    W = np.ascontiguousarray(np.asarray(W, dtype=np.float32))
    b = np.ascontiguousarray(np.asarray(b, dtype=np.float32))
    W16 = _hi16(W)
    maps = []
    for c in range(N_CORES):
        xc = x[c * ROWS:(c + 1) * ROWS]
        # [it, r, kt, p] -> [it, p, kt, r]: row-tile blocks, k-major inside
        blk = np.ascontiguousarray(
            xc.reshape(I_TILES, P, K_TILES, P).transpose(0, 3, 2, 1))
        maps.append({
            "xT": _hi16(blk).reshape(I_TILES * P, K_TILES * P),
            "W": W16,
            "b": b,
        })
    return maps


def kernel(x, W, b):
    from concourse.bass_utils import run_bass_kernel_spmd

    if "nc" not in _cached:
        _cached["nc"] = _build()
    nc = _cached["nc"]

    in_maps = make_in_maps(x, W, b)
    res = run_bass_kernel_spmd(nc, in_maps, list(range(N_CORES)))
    out = np.concatenate([res.results[c]["out"] for c in range(N_CORES)], axis=0)
    return out.astype(np.float32, copy=False)


# revision 15
# speedup vs baseline: 1.0141x; 1.0141x over previous
"""Trainium2 Bass kernel for nn_Dense_BinaryLayer (binary-weight dense layer).

out = x @ Wb + b, where Wb = binarize(W) in {-1, +1}.

Strategy: data-parallel over the 8 NeuronCores — each core handles 2048 rows
of x and the full (replicated) W and b; no collectives.  Host-side prep is
pure data movement: each core's x slice is permuted into per-row-tile
k-major blocks ([it, p, kt, r] layout, so every DMA is one contiguous
256 KiB DRAM read with 2 KiB per-partition segments), and both x and W are
passed as the high 16 bits of each f32 (byte-slice view = bf16 truncation,
no arithmetic; rel err ~3e-3 vs the 2e-2 gate, verified).  fp8 DoubleRow
was tried and measured: per-instruction time on HW equals bf16's, so the
exact 2-pass fp8 split is a wash — single-pass bf16 is the PE floor
(~57.5us busy).

Schedule (the PE stream executes in order, round-robin over 4 PSUM tiles,
so early tiles must land in consumption order):
  - W streams as 8 contiguous per-k-tile chunks spread over BOTH HWDGE
    rings so every wb k-tile is binarized by ~12us; DVE binarizes each
    chunk as it lands (bf16 2x DVE rate).
  - x row-tiles 0/1 go first on the SP HWDGE ring; the rest stream via
    SWDGE (x14/15 on the Act ring to balance), each with a dedicated SBUF
    buffer (no recycling stalls).
  - bf16 matmuls (free dim 512) accumulate in PSUM over the 8 k-tiles;
    DVE adds the broadcast bias on eviction; per-row-tile stores rotate
    across the three DMA rings.
"""
import sys

sys.path.insert(0, "/opt/trn_rl_repo")

import numpy as np

N_TOTAL = 16384
D_IN = 1024
D_OUT = 1024
N_CORES = 8
ROWS = N_TOTAL // N_CORES      # 2048 rows per core
P = 128
K_TILES = D_IN // P            # 8
I_TILES = ROWS // P            # 16
BIN_THRESH = 2.0 ** -24

_cached = {}


def _build():
    import concourse.tile as tile
    from concourse import bacc, mybir

    f32 = mybir.dt.float32
    bf16 = mybir.dt.bfloat16
    TS = mybir.AluOpType

    nc = bacc.Bacc()
    xt_d = nc.declare_dram_parameter(
        "xT", [I_TILES * P, K_TILES * P], bf16, isOutput=False)
    w_d = nc.declare_dram_parameter("W", [D_IN, D_OUT], bf16, isOutput=False)
    b_d = nc.declare_dram_parameter("b", [D_OUT], f32, isOutput=False)
    o_d = nc.declare_dram_parameter("out", [ROWS, D_OUT], f32, isOutput=True)

    with tile.TileContext(nc) as tc:
        with (
            tc.tile_pool(name="const", bufs=1) as const,
            tc.tile_pool(name="wpool", bufs=1) as wpool,
            tc.tile_pool(name="xts", bufs=I_TILES) as xts,
            tc.tile_pool(name="outp", bufs=8) as outp,
            tc.tile_pool(name="pso", bufs=4, space="PSUM") as pso,
        ):
            xt_ap = xt_d[:].rearrange("(it p) (kt r) -> it p kt r", p=P, kt=K_TILES)
            w_ap = w_d[:].rearrange("(kt p) j -> p kt j", p=P)

            # PE warmup: dummy matmuls on a memset tile, issued before any
            # DMA dependency, so the Tensor engine p-state is fully ramped
            # (and its pipeline warm) when the first real matmul is ready
            wsrc = const.tile([P, P], bf16, tag="wsrc")
            nc.gpsimd.memset(wsrc[:], 0.0)
            dummy = pso.tile([P, D_OUT], f32, tag="pso", name="dummy")
            for _ in range(62):
                nc.tensor.matmul(dummy[:, 0:P], wsrc[:], wsrc[:],
                                 start=True, stop=True)

            w_raw = wpool.tile([P, K_TILES, D_OUT], bf16, tag="wraw")
            x_tiles = []
            for it in range(I_TILES):
                x_tiles.append(
                    xts.tile([P, K_TILES, P], bf16, tag="x", name=f"xt_{it}"))

            # earliest-needed data first on each ring; each HWDGE ring
            # sustains only ~170 GB/s wall, so x0/x1 lead the sync ring
            # while W leads the Act ring, W spread over both
            nc.sync.dma_start(x_tiles[0][:], xt_ap[0])
            nc.scalar.dma_start(w_raw[:, 0, :], w_ap[:, 0, :])
            nc.sync.dma_start(x_tiles[1][:], xt_ap[1])
            nc.scalar.dma_start(w_raw[:, 1, :], w_ap[:, 1, :])
            nc.sync.dma_start(w_raw[:, 2, :], w_ap[:, 2, :])
            nc.scalar.dma_start(w_raw[:, 3, :], w_ap[:, 3, :])
            nc.sync.dma_start(w_raw[:, 4, :], w_ap[:, 4, :])
            nc.scalar.dma_start(w_raw[:, 5, :], w_ap[:, 5, :])
            nc.sync.dma_start(w_raw[:, 6, :], w_ap[:, 6, :])
            nc.scalar.dma_start(w_raw[:, 7, :], w_ap[:, 7, :])

            # bias broadcast to all partitions
            bb = const.tile([P, D_OUT], f32, tag="bb")
            nc.sync.dma_start(bb[:], b_d[:].unsqueeze(0).partition_broadcast(P))

            # bulk x via SWDGE (in consumption order); last two on Act ring
            for it in range(2, I_TILES - 2):
                nc.gpsimd.dma_start(x_tiles[it][:], xt_ap[it])
            nc.scalar.dma_start(x_tiles[I_TILES - 2][:], xt_ap[I_TILES - 2])
            nc.scalar.dma_start(x_tiles[I_TILES - 1][:], xt_ap[I_TILES - 1])

            # binarize on DVE per k-tile (bf16 => 2x DVE rate):
            # m = (W > c) in {0,1}, then Wb = 2m-1 in {+-1}
            wb = wpool.tile([P, K_TILES, D_OUT], bf16, tag="wb")
            wm = wpool.tile([P, D_OUT], bf16, tag="wm")
            for kt in range(K_TILES):
                nc.vector.tensor_scalar(
                    wm[:], w_raw[:, kt, :], BIN_THRESH, None, TS.is_gt,
                )
                nc.vector.tensor_scalar(
                    wb[:, kt, :], wm[:], 2.0, 1.0, TS.mult, TS.subtract,
                )

            for it in range(I_TILES):
                src = x_tiles[it]
                ps_o = pso.tile([P, D_OUT], f32, tag="pso", name=f"pso_{it}")
                for kt in range(K_TILES):
                    first = kt == 0
                    last = kt == K_TILES - 1
                    nc.tensor.matmul(
                        ps_o[:, 0:512],
                        src[:, kt, :],
                        wb[:, kt, 0:512],
                        start=first, stop=last,
                    )
                    nc.tensor.matmul(
                        ps_o[:, 512:1024],
                        src[:, kt, :],
                        wb[:, kt, 512:1024],
                        start=first, stop=last,
                    )
                # evict + store; the last two row-tiles go in quarters so
                # stores overlap evictions and the kernel tail shortens
                out_sb = outp.tile([P, D_OUT], f32, tag="out", name=f"out_{it}")
                n_chunks = 4 if it >= I_TILES - 2 else 1
                w_c = D_OUT // n_chunks
                for h in range(n_chunks):
                    cols = slice(h * w_c, (h + 1) * w_c)
                    nc.vector.tensor_tensor(
                        out=out_sb[:, cols], in0=ps_o[:, cols],
                        in1=bb[:, cols], op=TS.add,
                    )
                    ring = (nc.sync, nc.scalar, nc.gpsimd)[(it + h) % 3]
                    ring.dma_start(o_d[it * P:(it + 1) * P, cols],
                                   out_sb[:, cols])

    nc.compile()
    nc.finalize()
    return nc


def _hi16(a):
    """bf16 truncation of a C-contiguous f32 array as a byte-slice view."""
    import ml_dtypes

    u = a.view(np.uint16).reshape(*a.shape, 2)[..., 1]
    return np.ascontiguousarray(u).view(ml_dtypes.bfloat16)


def make_in_maps(x, W, b):
    x = np.ascontiguousarray(np.asarray(x, dtype=np.float32))
    W = np.ascontiguousarray(np.asarray(W, dtype=np.float32))
    b = np.ascontiguousarray(np.asarray(b, dtype=np.float32))
    W16 = _hi16(W)
    maps = []
    for c in range(N_CORES):
        xc = x[c * ROWS:(c + 1) * ROWS]
        # [it, r, kt, p] -> [it, p, kt, r]: row-tile blocks, k-major inside
        blk = np.ascontiguousarray(
            xc.reshape(I_TILES, P, K_TILES, P).transpose(0, 3, 2, 1))
        maps.append({
            "xT": _hi16(blk).reshape(I_TILES * P, K_TILES * P),
            "W": W16,
            "b": b,
        })
    return maps


def kernel(x, W, b):
    from concourse.bass_utils import run_bass_kernel_spmd

    if "nc" not in _cached:
        _cached["nc"] = _build()
    nc = _cached["nc"]

    in_maps = make_in_maps(x, W, b)
    res = run_bass_kernel_spmd(nc, in_maps, list(range(N_CORES)))
    out = np.concatenate([res.results[c]["out"] for c in range(N_CORES)], axis=0)
    return out.astype(np.float32, copy=False)
